# revision 35
# baseline (speedup 1.0000x reference)
"""Trainium2 Bass kernel for nn_MinRNNPredictor (2-layer minGRU + FC head).

Sharding: data-parallel over batch — each of the 8 NeuronCores runs the
full network on one batch row (the recurrence is independent per row);
the small weight matrices are replicated. No collectives.

Per-core dataflow (all on-chip tensors in [feature, time] layout):
  x.T (cast + pre-transposed on host; plain contiguous DMA loads)
    -> GEMM0 (PE, fp32 PSUM): pre_z0, pre_h0  [H, Tc]
    -> gates (ScalarE sigmoid, DVE scalar_tensor_tensor)
    -> h0 via DVE TensorTensorScan along the free/time axis
    -> GEMM1 -> gates -> scan -> h1
    -> FC with h1 as the *stationary* operand, producing y in natural
       [time, feature] layout (no output transpose needed).

The kernel is PE-bound (the bf16 matmul roofline is ~382us/core), so
precision is spent where the 2e-2 rel-err gate allows: the z-gate GEMMs
run (partly) in fp8-e4m3 with perf_mode=DoubleRow — two k-tiles per PE
pass, a true 2x on this shape (HW-measured 215.8ns per 512-wide pass,
LDWEIGHTS fully hidden at FD=512):
  - layer-1 z-GEMM: all 8 k-tiles fp8 (4 DR passes instead of 8 MMs);
    h0 is re-cast bf16->fp8 by ScalarE Copy into a pair-sliceable tile.
  - layer-0 z-GEMM: k0,k1 as one fp8 DR pass + k2,k3 in bf16 finishing
    the same PSUM accumulation group (half-contraction fp8 adds only
    sqrt(1/2) of a full fp8 GEMM's noise).
  - h-tilde GEMMs and FC stay bf16: sigmoid damping makes z-path noise
    cheap, but h-path/FC noise lands directly on the output (fp8 there
    measured 3.5-4e-2, over the gate).
Measured end-to-end rel err: 1.57e-2 (numpy simulation matches HW to 4
decimals). y leaves the chip as bf16 (host upcasts): halves the output
drain; +0.1% RMS.

Matmul scheduling details (HW-measured):
  - back-to-back [128,512] MMs pitch 215.8ns; chained same-bank
    accumulation +5-8ns/MM; *interleaving two open accumulation groups*
    costs ~30ns/MM — chains are kept contiguous per PSUM bank.
  - PE warmup: ~40 zero MMs on a DMA'd tile ride out the HAM half-clock
    window during the initial weight DMAs.
Weights are staged m-major ([P, m, k, 128], host pre-arranged) and
DMA'd in first-PE-use order: the fill phase is DMA-bandwidth-bound, so
each output m-tile's whole k-chain arrives as one contiguous transfer
just ahead of its first matmul. Biases are pre-striped/broadcast on
host so each is one clean DMA.

The time axis is processed in chunks of 512 (one PSUM bank). The chunk
loop is software-pipelined: per iteration the PE runs GEMM0(i),
GEMM1(i-1) and FC(i-2), so the serial DVE scan chain of a chunk always
overlaps a full iteration of PE work instead of stalling the PE at
chunk boundaries.
"""

import os

# This kernel must run on the axon-tunneled NeuronCores. A host process may
# pin JAX_PLATFORMS=cpu for its own reference math; drop such a pin before
# jax is imported (via concourse) so jax.devices() still sees the cores.
_jp = os.environ.get("JAX_PLATFORMS")
if _jp is not None and "axon" not in _jp and "neuron" not in _jp:
    os.environ.pop("JAX_PLATFORMS", None)

from contextlib import ExitStack

import ml_dtypes
import numpy as np

import concourse.mybir as mybir
import concourse.tile as tile
from concourse import bacc, bass_utils

P = 128
B, T, DIN, H, DOUT = 8, 4096, 512, 1024, 512
TC = 512  # time-chunk = one PSUM bank of fp32

F32 = mybir.dt.float32
BF16 = mybir.dt.bfloat16
FP8 = mybir.dt.float8e4
ALU = mybir.AluOpType
ACTF = mybir.ActivationFunctionType

GEMM_W = ("Wz0", "Wh0", "Wz1", "Wh1", "Wfc")
WEIGHT_NAMES = ("Wz0", "bz0", "Wh0", "bh0", "Wz1", "bz1", "Wh1", "bh1", "Wfc", "bfc")


def build(t_total=T, tcc=TC):
    """Build the single-core Bass module (same NEFF runs SPMD on all cores)."""
    nchunk = t_total // tcc
    assert t_total % tcc == 0 and tcc % P == 0
    hsub = H // P

    nc = bacc.Bacc("TRN2", target_bir_lowering=False, debug=False, num_devices=B)
    # x pre-transposed on host to [DIN/P, P, T] so every [P, tcc] x.T tile
    # is a single clean contiguous-row DMA (no xbar transposes on chip).
    x_d = nc.dram_tensor("xT", [DIN // P, P, t_total], BF16, kind="ExternalInput").ap()
    # fp8 copy of the first 256 x-features, pair-sliceable for DoubleRow.
    x8_d = nc.dram_tensor("xT8", [P, 2, t_total], FP8, kind="ExternalInput").ap()
    w_d = {}
    for name, shape, dt in (
        # Gate weights in m-major layout [P, m, k, 128] (host pre-arranged)
        # so each output m-tile's full k-chain is one contiguous DMA and
        # the chunk-0/1 matmuls unblock per-m instead of per-whole-weight.
        ("Wz0", [P, H // P, DIN // P, P], BF16),
        # First half of the layer-0 z contraction also runs fp8 DoubleRow
        # (one pass covers k0,k1; k2,k3 finish in bf16 in the same PSUM
        # group). Half-contraction fp8 keeps the added noise at sqrt(1/2)
        # of a full fp8 gemm; total rel err ~1.6e-2 vs the 2e-2 gate.
        ("Wz0f8", [P, H // P, 2, P], FP8),
        ("Wh0", [P, H // P, DIN // P, P], BF16),
        # The layer-1 z-gate GEMM runs in fp8-e4m3 DoubleRow (2 k-tiles per
        # PE pass): sigmoid damping + the convex gate mixing keep the extra
        # quantization noise at ~1e-2 total rel err, half the 2e-2 gate.
        ("Wz1", [P, H // P, H // P, P], FP8),
        ("Wh1", [P, H // P, H // P, P], BF16),
        ("Wfc", [H, DOUT], BF16),
        # Biases pre-striped on host: [P, 6*hsub] columns are
        # [bz0, bh0, bz1, bh1, -bz0, -bz1] stripes of [P, hsub] each.
        ("bias_pack", [P, 6 * (H // P)], F32),
        # FC bias pre-broadcast across partitions on host.
        ("bfc_rep", [P, DOUT], F32),
        # Zero tile for PE warmup: DMA'd so warmup matmuls start right
        # after engine init instead of waiting on a DVE memset.
        ("warm_in", [P, P], BF16),
    ):
        w_d[name] = nc.dram_tensor(name, shape, dt, kind="ExternalInput").ap()
    # y leaves the chip as bf16 (host upcasts): halves output DMA traffic
    # and the end-of-kernel drain; adds only ~0.1% RMS rounding.
    y_d = nc.dram_tensor("y", [t_total, DOUT], BF16, kind="ExternalOutput").ap()

    with tile.TileContext(nc) as tc, ExitStack() as ctx:
        const = ctx.enter_context(tc.tile_pool(name="const", bufs=1))
        sb = ctx.enter_context(tc.tile_pool(name="sb", bufs=2))
        psum = ctx.enter_context(tc.tile_pool(name="psum", bufs=8, space="PSUM"))

        xT_tiles = {}
        x8_tiles = {}
        h0_tiles = {}
        h1_tiles = {}
        carry0 = [None] * hsub
        carry1 = [None] * hsub

        def emit_T(i):
            """Load the x.T tiles of chunk i (host pre-transposed). The fp8
            pair tile goes first: it feeds the leading DoubleRow pass of
            every z-chain."""
            t8 = sb.tile([P, 2, tcc], FP8, tag="xT8", bufs=3, name=f"xT8_{i}")
            nc.gpsimd.dma_start(t8[:], x8_d[:, :, i * tcc : (i + 1) * tcc])
            x8_tiles[i] = t8
            xT = []
            for dj in range(DIN // P):
                t_ = sb.tile([P, tcc], BF16, tag=f"xT{dj}", bufs=3, name=f"xT{dj}_{i}")
                nc.gpsimd.dma_start(t_[:], x_d[dj, :, i * tcc : (i + 1) * tcc])
                xT.append(t_)
            xT_tiles[i] = xT

        # PE warmup: the HAM clock gate holds the PE at half clock until it
        # has seen ~3.4us of sustained activity. The PE is idle waiting on
        # weight DMAs at kernel start anyway, so burn that window on zero
        # matmuls to arrive at the first real GEMM already at full clock.
        # The zero tile arrives by DMA (fires during engine init) so the
        # warmup isn't serialized behind a DVE memset.
        warm = const.tile([P, P], BF16, name="warm")
        nc.sync.dma_start(warm[:], w_d["warm_in"])
        wp = psum.tile([P, P], F32, tag="psum", name="warm_psum")
        for _ in range(40):
            nc.tensor.matmul(wp[:], lhsT=warm[:], rhs=warm[:])

        # Bias pack first: one tiny clean DMA, needed by the first gates.
        bias_sb = const.tile([P, 6 * hsub], F32, name="bias_sb")
        nc.gpsimd.dma_start(bias_sb[:], w_d["bias_pack"])
        bz0_sb = bias_sb[:, 0 * hsub : 1 * hsub]
        bh0_sb = bias_sb[:, 1 * hsub : 2 * hsub]
        bz1_sb = bias_sb[:, 2 * hsub : 3 * hsub]
        bh1_sb = bias_sb[:, 3 * hsub : 4 * hsub]
        nbz0_sb = bias_sb[:, 4 * hsub : 5 * hsub]
        nbz1_sb = bias_sb[:, 5 * hsub : 6 * hsub]

        # x chunk 0 next: it gates the very first GEMM.
        emit_T(0)

        # Resident weights, m-major [P, m, k, 128]: one contiguous-row DMA
        # per output m-tile, issued in first-PE-use order so the pipeline
        # fill is gated per-m-tile rather than per-whole-weight (the fill
        # phase is DMA-bandwidth-bound).
        def w_tile(name, k_dim, dt=BF16):
            t_ = const.tile([P, H // P, k_dim // P, P], dt, name=f"{name}_sb")
            return t_, w_d[name]

        wz0_sb, wz0_src = w_tile("Wz0", DIN)
        wh0_sb, wh0_src = w_tile("Wh0", DIN)
        wz1_sb, wz1_src = w_tile("Wz1", H, dt=FP8)
        wh1_sb, wh1_src = w_tile("Wh1", H)
        wfc_sb = const.tile([P, H // P, DOUT], BF16, name="Wfc_sb")
        wz0f8_sb = const.tile([P, H // P, 2, P], FP8, name="Wz0f8_sb")

        for m in range(hsub):
            nc.sync.dma_start(wz0f8_sb[:, m], w_d["Wz0f8"][:, m])
            nc.sync.dma_start(wz0_sb[:, m], wz0_src[:, m])
            nc.sync.dma_start(wh0_sb[:, m], wh0_src[:, m])
        for m in range(hsub):
            nc.sync.dma_start(wz1_sb[:, m], wz1_src[:, m])
            nc.sync.dma_start(wh1_sb[:, m], wh1_src[:, m])
        nc.sync.dma_start(
            wfc_sb[:], w_d["Wfc"].rearrange("(o p) n -> p o n", p=P)
        )

        # FC bias (pre-broadcast on host): one clean DMA, needed by FC(0)
        # ~100us in — emitted inside the loop to stay off the critical path.
        bfc_sb = const.tile([P, DOUT], F32, name="bfc_sb")

        def emit_layer(
            i, w_z, w_h, bz, nbz, bh, rhs_tiles, carry, out_tiles, ltag, z_dr=None
        ):
            ksub = len(rhs_tiles)
            outs = []
            for m in range(hsub):
                pz = psum.tile([P, tcc], F32, tag="psum", name=f"pz{ltag}_{i}_{m}")
                ph = psum.tile([P, tcc], F32, tag="psum", name=f"ph{ltag}_{i}_{m}")
                # Accumulation chains stay contiguous per bank: interleaving
                # two open groups costs ~30ns/MM (HW-measured), chained
                # same-bank accumulation only ~5ns/MM.
                if z_dr is not None:
                    # z-path fp8 DoubleRow passes (two k-tiles each), then
                    # any remaining k-tiles finish in bf16 in the same
                    # accumulation group.
                    w_zf8, rhs_f8, npairs = z_dr
                    for p in range(npairs):
                        nc.tensor.matmul(
                            pz[:],
                            lhsT=w_zf8[:, m, 2 * p : 2 * p + 2, :],
                            rhs=rhs_f8[:, 2 * p : 2 * p + 2, :],
                            start=(p == 0),
                            stop=(p == npairs - 1 and 2 * npairs == ksub),
                            perf_mode=mybir.MatmulPerfMode.DoubleRow,
                        )
                    for k in range(2 * npairs, ksub):
                        nc.tensor.matmul(
                            pz[:],
                            lhsT=w_z[:, m, k, :],
                            rhs=rhs_tiles[k][:],
                            start=False,
                            stop=(k == ksub - 1),
                        )
                else:
                    for k in range(ksub):
                        nc.tensor.matmul(
                            pz[:],
                            lhsT=w_z[:, m, k, :],
                            rhs=rhs_tiles[k][:],
                            start=(k == 0),
                            stop=(k == ksub - 1),
                        )
                for k in range(ksub):
                    nc.tensor.matmul(
                        ph[:],
                        lhsT=w_h[:, m, k, :],
                        rhs=rhs_tiles[k][:],
                        start=(k == 0),
                        stop=(k == ksub - 1),
                    )
                # a = 1 - z = sigmoid(-(pre_z + bz)); z = sigmoid(pre_z + bz)
                a_t = sb.tile(
                    [P, tcc], BF16, tag=f"a{ltag}", bufs=4, name=f"a{ltag}_{i}_{m}"
                )
                nc.scalar.activation(
                    a_t[:], pz[:], ACTF.Sigmoid, bias=nbz[:, m : m + 1], scale=-1.0
                )
                z_t = sb.tile(
                    [P, tcc], BF16, tag=f"z{ltag}", bufs=4, name=f"z{ltag}_{i}_{m}"
                )
                nc.scalar.activation(
                    z_t[:], pz[:], ACTF.Sigmoid, bias=bz[:, m : m + 1], scale=1.0
                )
                # b = (pre_h + bh) * z
                b_t = sb.tile(
                    [P, tcc], BF16, tag=f"b{ltag}", bufs=4, name=f"b{ltag}_{i}_{m}"
                )
                nc.vector.scalar_tensor_tensor(
                    b_t[:], ph[:], bh[:, m : m + 1], z_t[:], op0=ALU.add, op1=ALU.mult
                )
                # h_t = a_t * h_{t-1} + b_t along the time (free) axis
                h_t = sb.tile(
                    [P, tcc], BF16, tag=f"h{ltag}_{m}", bufs=3, name=f"h{ltag}_{i}_{m}"
                )
                init = 0.0 if carry[m] is None else carry[m][:, tcc - 1 : tcc]
                nc.vector.tensor_tensor_scan(
                    h_t[:], a_t[:], b_t[:], init, op0=ALU.mult, op1=ALU.add
                )
                carry[m] = h_t
                outs.append(h_t)
            out_tiles[i] = outs

        def emit_FC(i):
            h1 = h1_tiles.pop(i)
            for tt in range(tcc // P):
                yp = psum.tile([P, DOUT], F32, tag="psum", name=f"yp_{i}_{tt}")
                for j in range(hsub):
                    nc.tensor.matmul(
                        yp[:],
                        lhsT=h1[j][:, tt * P : (tt + 1) * P],
                        rhs=wfc_sb[:, j, :],
                        start=(j == 0),
                        stop=(j == hsub - 1),
                    )
                y_sb = sb.tile([P, DOUT], BF16, tag="y", bufs=4, name=f"y_{i}_{tt}")
                nc.vector.tensor_tensor(y_sb[:], yp[:], bfc_sb[:], ALU.add)
                t0 = i * tcc + tt * P
                nc.sync.dma_start(y_d[t0 : t0 + P, :], y_sb[:])

        h0f8_tiles = {}

        def emit_casts(i):
            """bf16 h0 -> fp8 copy (ScalarE) into one pair-sliceable tile for
            the next iteration's DoubleRow z-GEMM."""
            t_ = sb.tile([P, hsub, tcc], FP8, tag="h0f8", bufs=2, name=f"h0f8_{i}")
            for m in range(hsub):
                nc.scalar.activation(
                    t_[:, m, :], h0_tiles[i][m][:], ACTF.Copy, scale=1.0
                )
            h0f8_tiles[i] = t_

        # Software-pipelined chunk loop (stages offset on the PE stream).
        for i in range(nchunk + 2):
            if i < nchunk:
                emit_layer(
                    i, wz0_sb, wh0_sb, bz0_sb, nbz0_sb, bh0_sb,
                    xT_tiles.pop(i), carry0, h0_tiles, "0",
                    z_dr=(wz0f8_sb, x8_tiles.pop(i), 1),
                )
            if i == 1:
                nc.sync.dma_start(bfc_sb[:], w_d["bfc_rep"])
            if i + 1 < nchunk:
                emit_T(i + 1)
            if 1 <= i <= nchunk:
                emit_layer(
                    i - 1, wz1_sb, wh1_sb, bz1_sb, nbz1_sb, bh1_sb,
                    h0_tiles.pop(i - 1), carry1, h1_tiles, "1",
                    z_dr=(wz1_sb, h0f8_tiles.pop(i - 1), (H // P) // 2),
                )
            if 2 <= i <= nchunk + 1:
                emit_FC(i - 2)
            if i < nchunk:
                emit_casts(i)

    nc.compile()
    return nc


_NC_CACHE = {}


def _get_nc(t_total=T, tcc=TC):
    key = (t_total, tcc)
    if key not in _NC_CACHE:
        _NC_CACHE[key] = build(t_total, tcc)
    return _NC_CACHE[key]


_RUNNER = None


def _get_runner():
    """Build (once) a cached jitted SPMD executor for the module so repeated
    kernel() calls reuse the compiled NEFF instead of re-tracing."""
    global _RUNNER
    if _RUNNER is None:
        import jax
        from jax.experimental.shard_map import shard_map
        from jax.sharding import Mesh, PartitionSpec

        from concourse import bass2jax

        bass2jax.install_neuronx_cc_hook()
        nc = _get_nc()
        assert nc.dbg_addr is None
        partition_name = (
            nc.partition_id_tensor.name if nc.partition_id_tensor else None
        )
        in_names, out_names, out_avals = [], [], []
        for alloc in nc.m.functions[0].allocations:
            if not isinstance(alloc, mybir.MemoryLocationSet):
                continue
            name = alloc.memorylocations[0].name
            if alloc.kind == "ExternalInput":
                if name != partition_name:
                    in_names.append(name)
            elif alloc.kind == "ExternalOutput":
                out_names.append(name)
                out_avals.append(
                    jax.core.ShapedArray(
                        tuple(alloc.tensor_shape), mybir.dt.np(alloc.dtype)
                    )
                )
        n_params = len(in_names)
        n_outs = len(out_names)
        all_names = tuple(in_names) + tuple(out_names)
        if partition_name is not None:
            all_names = all_names + (partition_name,)

        def _body(*args):
            operands = list(args)
            if partition_name is not None:
                operands.append(bass2jax.partition_id_tensor())
            outs = bass2jax._bass_exec_p.bind(
                *operands,
                out_avals=tuple(out_avals),
                in_names=all_names,
                out_names=tuple(out_names),
                lowering_input_output_aliases=(),
                sim_require_finite=True,
                sim_require_nnan=True,
                nc=nc,
            )
            return tuple(outs)

        devices = jax.devices()[:B]
        assert len(devices) == B, f"need {B} cores, found {len(jax.devices())}"
        mesh = Mesh(np.asarray(devices), ("core",))
        sharded = jax.jit(
            shard_map(
                _body,
                mesh=mesh,
                in_specs=(PartitionSpec("core"),) * (n_params + n_outs),
                out_specs=(PartitionSpec("core"),) * n_outs,
                check_rep=False,
            ),
            donate_argnums=tuple(range(n_params, n_params + n_outs)),
            keep_unused=True,
        )
        _RUNNER = (sharded, list(in_names), list(out_names), list(out_avals))
    return _RUNNER


def pack_biases(inputs):
    """Host-side bias staging: stripe gate biases to [P, 6*hsub] (including
    negated z-biases) and broadcast bfc to [P, DOUT]."""
    hsub = H // P

    def stripe(name):
        return np.asarray(inputs[name], np.float32).reshape(hsub, P).T

    pack = np.concatenate(
        [
            stripe("bz0"), stripe("bh0"), stripe("bz1"), stripe("bh1"),
            -stripe("bz0"), -stripe("bz1"),
        ],
        axis=1,
    )
    bfc_rep = np.broadcast_to(
        np.asarray(inputs["bfc"], np.float32), (P, DOUT)
    )
    return {
        "bias_pack": np.ascontiguousarray(pack),
        "bfc_rep": np.ascontiguousarray(bfc_rep),
    }


def run(inputs, trace=False, **spmd_kwargs):
    """Run the SPMD kernel on all 8 cores. Returns (y[B,T,DOUT], results)."""
    x = np.asarray(inputs["x"], dtype=np.float32)
    assert x.shape == (B, T, DIN), x.shape
    # [B, T, DIN] -> per-core [DIN/P, P, T] bf16 (cast + transpose staging)
    x_bf = np.ascontiguousarray(
        x.astype(ml_dtypes.bfloat16).transpose(0, 2, 1).reshape(B, DIN // P, P, T)
    )
    # fp8 copy of features 0..255, [B, P, 2, T], pair-sliceable on chip
    x_f8 = np.ascontiguousarray(
        x[:, :, : 2 * P]
        .astype(ml_dtypes.float8_e4m3)
        .transpose(0, 2, 1)
        .reshape(B, 2, P, T)
        .transpose(0, 2, 1, 3)
    )
    shared = {}
    for name in GEMM_W:
        host_dt = ml_dtypes.float8_e4m3 if name == "Wz1" else ml_dtypes.bfloat16
        w = np.asarray(inputs[name], dtype=np.float32).astype(host_dt)
        if name != "Wfc":
            # [K, N] -> m-major [P, m, k, 128]
            kd, nd = w.shape
            w = w.reshape(kd // P, P, nd // P, P).transpose(1, 2, 0, 3)
        shared[name] = np.ascontiguousarray(w)
    wz0f8 = np.asarray(inputs["Wz0"], dtype=np.float32)[: 2 * P].astype(
        ml_dtypes.float8_e4m3
    )
    shared["Wz0f8"] = np.ascontiguousarray(
        wz0f8.reshape(2, P, H // P, P).transpose(1, 2, 0, 3)
    )
    shared.update(pack_biases(inputs))
    shared["warm_in"] = np.zeros((P, P), dtype=ml_dtypes.bfloat16)
    in_maps = [dict(shared, xT=x_bf[c], xT8=x_f8[c]) for c in range(B)]
    if trace or spmd_kwargs:
        nc = _get_nc()
        res = bass_utils.run_bass_kernel_spmd(
            nc, in_maps, core_ids=list(range(B)), trace=trace, **spmd_kwargs
        )
        y = np.stack([r["y"] for r in res.results], axis=0).astype(np.float32)
        return y, res
    sharded, in_names, out_names, out_avals = _get_runner()
    per_core = [[np.asarray(m[n]) for n in in_names] for m in in_maps]
    concat_in = [
        np.concatenate([per_core[c][i] for c in range(B)], axis=0)
        for i in range(len(in_names))
    ]
    concat_zeros = [
        np.zeros((B * a.shape[0], *a.shape[1:]), a.dtype) for a in out_avals
    ]
    outs = sharded(*concat_in, *concat_zeros)
    yi = out_names.index("y")
    y = np.asarray(outs[yi]).reshape(B, *out_avals[yi].shape).astype(np.float32)
    return y, None


def kernel(**inputs) -> np.ndarray:
    y, _ = run(inputs)
    return y



# revision 39
# speedup vs baseline: 1.0085x; 1.0085x over previous
"""Trainium2 Bass kernel for nn_MinRNNPredictor (2-layer minGRU + FC head).

Sharding: data-parallel over batch — each of the 8 NeuronCores runs the
full network on one batch row (the recurrence is independent per row);
the small weight matrices are replicated. No collectives.

Per-core dataflow (all on-chip tensors in [feature, time] layout):
  x.T (cast + pre-transposed on host; plain contiguous DMA loads)
    -> GEMM0 (PE, fp32 PSUM): pre_z0, pre_h0  [H, Tc]
    -> gates (ScalarE sigmoid, DVE scalar_tensor_tensor)
    -> h0 via DVE TensorTensorScan along the free/time axis
    -> GEMM1 -> gates -> scan -> h1
    -> FC with h1 as the *stationary* operand, producing y in natural
       [time, feature] layout (no output transpose needed).

The kernel is PE-bound (the bf16 matmul roofline is ~382us/core), so
precision is spent where the 2e-2 rel-err gate allows: the z-gate GEMMs
run (partly) in fp8-e4m3 with perf_mode=DoubleRow — two k-tiles per PE
pass, a true 2x on this shape (HW-measured 215.8ns per 512-wide pass,
LDWEIGHTS fully hidden at FD=512):
  - layer-1 z-GEMM: all 8 k-tiles fp8 (4 DR passes instead of 8 MMs);
    h0 is re-cast bf16->fp8 by ScalarE Copy into a pair-sliceable tile.
  - layer-0 z-GEMM: k0,k1 as one fp8 DR pass + k2,k3 in bf16 finishing
    the same PSUM accumulation group (half-contraction fp8 adds only
    sqrt(1/2) of a full fp8 GEMM's noise).
  - h-tilde GEMMs and FC stay bf16: sigmoid damping makes z-path noise
    cheap, but h-path/FC noise lands directly on the output (fp8 there
    measured 3.5-4e-2, over the gate).
Measured end-to-end rel err: 1.57e-2 (numpy simulation matches HW to 4
decimals). y leaves the chip as bf16 (host upcasts): halves the output
drain; +0.1% RMS.

Matmul scheduling details (HW-measured):
  - back-to-back [128,512] MMs pitch 215.8ns; chained same-bank
    accumulation +5-8ns/MM; *interleaving two open accumulation groups*
    costs ~30ns/MM — chains are kept contiguous per PSUM bank.
  - PE warmup: ~40 zero MMs on a DMA'd tile ride out the HAM half-clock
    window during the initial weight DMAs.
Weights are staged m-major ([P, m, k, 128], host pre-arranged) and
DMA'd in first-PE-use order: the fill phase is DMA-bandwidth-bound, so
each output m-tile's whole k-chain arrives as one contiguous transfer
just ahead of its first matmul. Biases are pre-striped/broadcast on
host so each is one clean DMA.

The time axis is processed in chunks of 512 (one PSUM bank). The chunk
loop is software-pipelined: per iteration the PE runs GEMM0(i),
GEMM1(i-1) and FC(i-2), so the serial DVE scan chain of a chunk always
overlaps a full iteration of PE work instead of stalling the PE at
chunk boundaries.
"""

import os

# This kernel must run on the axon-tunneled NeuronCores. A host process may
# pin JAX_PLATFORMS=cpu for its own reference math; drop such a pin before
# jax is imported (via concourse) so jax.devices() still sees the cores.
_jp = os.environ.get("JAX_PLATFORMS")
if _jp is not None and "axon" not in _jp and "neuron" not in _jp:
    os.environ.pop("JAX_PLATFORMS", None)

from contextlib import ExitStack

import ml_dtypes
import numpy as np

import concourse.mybir as mybir
import concourse.tile as tile
from concourse import bacc, bass_utils

P = 128
B, T, DIN, H, DOUT = 8, 4096, 512, 1024, 512
TC = 512  # time-chunk = one PSUM bank of fp32

F32 = mybir.dt.float32
BF16 = mybir.dt.bfloat16
FP8 = mybir.dt.float8e4
ALU = mybir.AluOpType
ACTF = mybir.ActivationFunctionType

GEMM_W = ("Wz0", "Wh0", "Wz1", "Wh1", "Wfc")
WEIGHT_NAMES = ("Wz0", "bz0", "Wh0", "bh0", "Wz1", "bz1", "Wh1", "bh1", "Wfc", "bfc")


def build(t_total=T, tcc=TC):
    """Build the single-core Bass module (same NEFF runs SPMD on all cores)."""
    nchunk = t_total // tcc
    assert t_total % tcc == 0 and tcc % P == 0
    hsub = H // P

    nc = bacc.Bacc("TRN2", target_bir_lowering=False, debug=False, num_devices=B)
    # x pre-transposed on host to [DIN/P, P, T] so every [P, tcc] x.T tile
    # is a single clean contiguous-row DMA (no xbar transposes on chip).
    x_d = nc.dram_tensor("xT", [DIN // P, P, t_total], BF16, kind="ExternalInput").ap()
    # fp8 copy of the first 256 x-features, pair-sliceable for DoubleRow.
    x8_d = nc.dram_tensor("xT8", [P, 2, t_total], FP8, kind="ExternalInput").ap()
    w_d = {}
    for name, shape, dt in (
        # Gate weights in m-major layout [P, m, k, 128] (host pre-arranged)
        # so each output m-tile's full k-chain is one contiguous DMA and
        # the chunk-0/1 matmuls unblock per-m instead of per-whole-weight.
        ("Wz0", [P, H // P, DIN // P, P], BF16),
        # First half of the layer-0 z contraction also runs fp8 DoubleRow
        # (one pass covers k0,k1; k2,k3 finish in bf16 in the same PSUM
        # group). Half-contraction fp8 keeps the added noise at sqrt(1/2)
        # of a full fp8 gemm; total rel err ~1.6e-2 vs the 2e-2 gate.
        ("Wz0f8", [P, H // P, 2, P], FP8),
        ("Wh0", [P, H // P, DIN // P, P], BF16),
        # The layer-1 z-gate GEMM runs in fp8-e4m3 DoubleRow (2 k-tiles per
        # PE pass): sigmoid damping + the convex gate mixing keep the extra
        # quantization noise at ~1e-2 total rel err, half the 2e-2 gate.
        ("Wz1", [P, H // P, H // P, P], FP8),
        ("Wh1", [P, H // P, H // P, P], BF16),
        ("Wfc", [H, DOUT], BF16),
        # Biases pre-striped on host: [P, 6*hsub] columns are
        # [bz0, bh0, bz1, bh1, -bz0, -bz1] stripes of [P, hsub] each.
        ("bias_pack", [P, 6 * (H // P)], F32),
        # FC bias pre-broadcast across partitions on host.
        ("bfc_rep", [P, DOUT], F32),
        # Zero tile for PE warmup: DMA'd so warmup matmuls start right
        # after engine init instead of waiting on a DVE memset.
        ("warm_in", [P, P], BF16),
    ):
        w_d[name] = nc.dram_tensor(name, shape, dt, kind="ExternalInput").ap()
    # y leaves the chip as bf16 (host upcasts): halves output DMA traffic
    # and the end-of-kernel drain; adds only ~0.1% RMS rounding.
    y_d = nc.dram_tensor("y", [t_total, DOUT], BF16, kind="ExternalOutput").ap()

    with tile.TileContext(nc) as tc, ExitStack() as ctx:
        const = ctx.enter_context(tc.tile_pool(name="const", bufs=1))
        sb = ctx.enter_context(tc.tile_pool(name="sb", bufs=2))
        psum = ctx.enter_context(tc.tile_pool(name="psum", bufs=8, space="PSUM"))

        xT_tiles = {}
        x8_tiles = {}
        h0_tiles = {}
        h1_tiles = {}
        carry0 = [None] * hsub
        carry1 = [None] * hsub

        def emit_T(i):
            """Load the x.T tiles of chunk i (host pre-transposed). The fp8
            pair tile goes first: it feeds the leading DoubleRow pass of
            every z-chain."""
            t8 = sb.tile([P, 2, tcc], FP8, tag="xT8", bufs=3, name=f"xT8_{i}")
            nc.gpsimd.dma_start(t8[:], x8_d[:, :, i * tcc : (i + 1) * tcc])
            x8_tiles[i] = t8
            xT = []
            for dj in range(DIN // P):
                t_ = sb.tile([P, tcc], BF16, tag=f"xT{dj}", bufs=3, name=f"xT{dj}_{i}")
                nc.gpsimd.dma_start(t_[:], x_d[dj, :, i * tcc : (i + 1) * tcc])
                xT.append(t_)
            xT_tiles[i] = xT

        # PE warmup: the HAM clock gate holds the PE at half clock until it
        # has seen ~3.4us of sustained activity. The PE is idle waiting on
        # weight DMAs at kernel start anyway, so burn that window on zero
        # matmuls to arrive at the first real GEMM already at full clock.
        # The zero tile arrives by DMA (fires during engine init) so the
        # warmup isn't serialized behind a DVE memset.
        warm = const.tile([P, P], BF16, name="warm")
        nc.sync.dma_start(warm[:], w_d["warm_in"])
        wp = psum.tile([P, P], F32, tag="psum", name="warm_psum")
        for _ in range(40):
            nc.tensor.matmul(wp[:], lhsT=warm[:], rhs=warm[:])

        # Bias pack first: one tiny clean DMA, needed by the first gates.
        bias_sb = const.tile([P, 6 * hsub], F32, name="bias_sb")
        nc.gpsimd.dma_start(bias_sb[:], w_d["bias_pack"])
        bz0_sb = bias_sb[:, 0 * hsub : 1 * hsub]
        bh0_sb = bias_sb[:, 1 * hsub : 2 * hsub]
        bz1_sb = bias_sb[:, 2 * hsub : 3 * hsub]
        bh1_sb = bias_sb[:, 3 * hsub : 4 * hsub]
        nbz0_sb = bias_sb[:, 4 * hsub : 5 * hsub]
        nbz1_sb = bias_sb[:, 5 * hsub : 6 * hsub]

        # x chunk 0 next: it gates the very first GEMM.
        emit_T(0)

        # Resident weights, m-major [P, m, k, 128]: one contiguous-row DMA
        # per output m-tile, issued in first-PE-use order so the pipeline
        # fill is gated per-m-tile rather than per-whole-weight (the fill
        # phase is DMA-bandwidth-bound).
        def w_tile(name, k_dim, dt=BF16):
            t_ = const.tile([P, H // P, k_dim // P, P], dt, name=f"{name}_sb")
            return t_, w_d[name]

        wz0_sb, wz0_src = w_tile("Wz0", DIN)
        wh0_sb, wh0_src = w_tile("Wh0", DIN)
        wz1_sb, wz1_src = w_tile("Wz1", H, dt=FP8)
        wh1_sb, wh1_src = w_tile("Wh1", H)
        wfc_sb = const.tile([P, H // P, DOUT], BF16, name="Wfc_sb")
        wz0f8_sb = const.tile([P, H // P, 2, P], FP8, name="Wz0f8_sb")

        for m in range(hsub):
            nc.sync.dma_start(wz0f8_sb[:, m], w_d["Wz0f8"][:, m])
            # k0,k1 of bf16 Wz0 are covered by the fp8 DoubleRow pass and
            # never read — only load the k2,k3 half of each m-block.
            nc.sync.dma_start(wz0_sb[:, m, 2:4], wz0_src[:, m, 2:4])
            nc.sync.dma_start(wh0_sb[:, m], wh0_src[:, m])
        for m in range(hsub):
            nc.sync.dma_start(wz1_sb[:, m], wz1_src[:, m])
            nc.sync.dma_start(wh1_sb[:, m], wh1_src[:, m])
        nc.sync.dma_start(
            wfc_sb[:], w_d["Wfc"].rearrange("(o p) n -> p o n", p=P)
        )

        # FC bias (pre-broadcast on host): one clean DMA, needed by FC(0)
        # ~100us in — emitted inside the loop to stay off the critical path.
        bfc_sb = const.tile([P, DOUT], F32, name="bfc_sb")

        def emit_layer(
            i, w_z, w_h, bz, nbz, bh, rhs_tiles, carry, out_tiles, ltag, z_dr=None
        ):
            ksub = len(rhs_tiles)
            outs = []
            for m in range(hsub):
                pz = psum.tile([P, tcc], F32, tag="psum", name=f"pz{ltag}_{i}_{m}")
                ph = psum.tile([P, tcc], F32, tag="psum", name=f"ph{ltag}_{i}_{m}")
                # Accumulation chains stay contiguous per bank: interleaving
                # two open groups costs ~30ns/MM (HW-measured), chained
                # same-bank accumulation only ~5ns/MM.
                if z_dr is not None:
                    # z-path fp8 DoubleRow passes (two k-tiles each), then
                    # any remaining k-tiles finish in bf16 in the same
                    # accumulation group.
                    w_zf8, rhs_f8, npairs = z_dr
                    for p in range(npairs):
                        nc.tensor.matmul(
                            pz[:],
                            lhsT=w_zf8[:, m, 2 * p : 2 * p + 2, :],
                            rhs=rhs_f8[:, 2 * p : 2 * p + 2, :],
                            start=(p == 0),
                            stop=(p == npairs - 1 and 2 * npairs == ksub),
                            perf_mode=mybir.MatmulPerfMode.DoubleRow,
                        )
                    for k in range(2 * npairs, ksub):
                        nc.tensor.matmul(
                            pz[:],
                            lhsT=w_z[:, m, k, :],
                            rhs=rhs_tiles[k][:],
                            start=False,
                            stop=(k == ksub - 1),
                        )
                else:
                    for k in range(ksub):
                        nc.tensor.matmul(
                            pz[:],
                            lhsT=w_z[:, m, k, :],
                            rhs=rhs_tiles[k][:],
                            start=(k == 0),
                            stop=(k == ksub - 1),
                        )
                for k in range(ksub):
                    nc.tensor.matmul(
                        ph[:],
                        lhsT=w_h[:, m, k, :],
                        rhs=rhs_tiles[k][:],
                        start=(k == 0),
                        stop=(k == ksub - 1),
                    )
                # a = 1 - z = sigmoid(-(pre_z + bz)); z = sigmoid(pre_z + bz)
                a_t = sb.tile(
                    [P, tcc], BF16, tag=f"a{ltag}", bufs=4, name=f"a{ltag}_{i}_{m}"
                )
                nc.scalar.activation(
                    a_t[:], pz[:], ACTF.Sigmoid, bias=nbz[:, m : m + 1], scale=-1.0
                )
                z_t = sb.tile(
                    [P, tcc], BF16, tag=f"z{ltag}", bufs=4, name=f"z{ltag}_{i}_{m}"
                )
                nc.scalar.activation(
                    z_t[:], pz[:], ACTF.Sigmoid, bias=bz[:, m : m + 1], scale=1.0
                )
                # b = (pre_h + bh) * z
                b_t = sb.tile(
                    [P, tcc], BF16, tag=f"b{ltag}", bufs=4, name=f"b{ltag}_{i}_{m}"
                )
                nc.vector.scalar_tensor_tensor(
                    b_t[:], ph[:], bh[:, m : m + 1], z_t[:], op0=ALU.add, op1=ALU.mult
                )
                # h_t = a_t * h_{t-1} + b_t along the time (free) axis
                h_t = sb.tile(
                    [P, tcc], BF16, tag=f"h{ltag}_{m}", bufs=3, name=f"h{ltag}_{i}_{m}"
                )
                init = 0.0 if carry[m] is None else carry[m][:, tcc - 1 : tcc]
                nc.vector.tensor_tensor_scan(
                    h_t[:], a_t[:], b_t[:], init, op0=ALU.mult, op1=ALU.add
                )
                carry[m] = h_t
                outs.append(h_t)
            out_tiles[i] = outs

        def emit_FC(i):
            h1 = h1_tiles.pop(i)
            for tt in range(tcc // P):
                yp = psum.tile([P, DOUT], F32, tag="psum", name=f"yp_{i}_{tt}")
                for j in range(hsub):
                    nc.tensor.matmul(
                        yp[:],
                        lhsT=h1[j][:, tt * P : (tt + 1) * P],
                        rhs=wfc_sb[:, j, :],
                        start=(j == 0),
                        stop=(j == hsub - 1),
                    )
                y_sb = sb.tile([P, DOUT], BF16, tag="y", bufs=4, name=f"y_{i}_{tt}")
                nc.vector.tensor_tensor(y_sb[:], yp[:], bfc_sb[:], ALU.add)
                t0 = i * tcc + tt * P
                if i == nchunk - 1:
                    # Final chunk: split each y store across two DMA issues
                    # so the end-of-kernel drain spreads over more rings.
                    nc.sync.dma_start(
                        y_d[t0 : t0 + P, : DOUT // 2], y_sb[:, : DOUT // 2]
                    )
                    nc.sync.dma_start(
                        y_d[t0 : t0 + P, DOUT // 2 :], y_sb[:, DOUT // 2 :]
                    )
                else:
                    nc.sync.dma_start(y_d[t0 : t0 + P, :], y_sb[:])

        h0f8_tiles = {}

        def emit_casts(i):
            """bf16 h0 -> fp8 copy (ScalarE) into one pair-sliceable tile for
            the next iteration's DoubleRow z-GEMM."""
            t_ = sb.tile([P, hsub, tcc], FP8, tag="h0f8", bufs=2, name=f"h0f8_{i}")
            for m in range(hsub):
                nc.scalar.activation(
                    t_[:, m, :], h0_tiles[i][m][:], ACTF.Copy, scale=1.0
                )
            h0f8_tiles[i] = t_

        # Software-pipelined chunk loop (stages offset on the PE stream).
        for i in range(nchunk + 2):
            if i < nchunk:
                emit_layer(
                    i, wz0_sb, wh0_sb, bz0_sb, nbz0_sb, bh0_sb,
                    xT_tiles.pop(i), carry0, h0_tiles, "0",
                    z_dr=(wz0f8_sb, x8_tiles.pop(i), 1),
                )
            if i == 1:
                nc.sync.dma_start(bfc_sb[:], w_d["bfc_rep"])
            if i + 1 < nchunk:
                emit_T(i + 1)
            if 1 <= i <= nchunk:
                emit_layer(
                    i - 1, wz1_sb, wh1_sb, bz1_sb, nbz1_sb, bh1_sb,
                    h0_tiles.pop(i - 1), carry1, h1_tiles, "1",
                    z_dr=(wz1_sb, h0f8_tiles.pop(i - 1), (H // P) // 2),
                )
            if 2 <= i <= nchunk + 1:
                emit_FC(i - 2)
            if i < nchunk:
                emit_casts(i)

    nc.compile()
    return nc


_NC_CACHE = {}


def _get_nc(t_total=T, tcc=TC):
    key = (t_total, tcc)
    if key not in _NC_CACHE:
        _NC_CACHE[key] = build(t_total, tcc)
    return _NC_CACHE[key]


_RUNNER = None


def _get_runner():
    """Build (once) a cached jitted SPMD executor for the module so repeated
    kernel() calls reuse the compiled NEFF instead of re-tracing."""
    global _RUNNER
    if _RUNNER is None:
        import jax
        from jax.experimental.shard_map import shard_map
        from jax.sharding import Mesh, PartitionSpec

        from concourse import bass2jax

        bass2jax.install_neuronx_cc_hook()
        nc = _get_nc()
        assert nc.dbg_addr is None
        partition_name = (
            nc.partition_id_tensor.name if nc.partition_id_tensor else None
        )
        in_names, out_names, out_avals = [], [], []
        for alloc in nc.m.functions[0].allocations:
            if not isinstance(alloc, mybir.MemoryLocationSet):
                continue
            name = alloc.memorylocations[0].name
            if alloc.kind == "ExternalInput":
                if name != partition_name:
                    in_names.append(name)
            elif alloc.kind == "ExternalOutput":
                out_names.append(name)
                out_avals.append(
                    jax.core.ShapedArray(
                        tuple(alloc.tensor_shape), mybir.dt.np(alloc.dtype)
                    )
                )
        n_params = len(in_names)
        n_outs = len(out_names)
        all_names = tuple(in_names) + tuple(out_names)
        if partition_name is not None:
            all_names = all_names + (partition_name,)

        def _body(*args):
            operands = list(args)
            if partition_name is not None:
                operands.append(bass2jax.partition_id_tensor())
            outs = bass2jax._bass_exec_p.bind(
                *operands,
                out_avals=tuple(out_avals),
                in_names=all_names,
                out_names=tuple(out_names),
                lowering_input_output_aliases=(),
                sim_require_finite=True,
                sim_require_nnan=True,
                nc=nc,
            )
            return tuple(outs)

        devices = jax.devices()[:B]
        assert len(devices) == B, f"need {B} cores, found {len(jax.devices())}"
        mesh = Mesh(np.asarray(devices), ("core",))
        sharded = jax.jit(
            shard_map(
                _body,
                mesh=mesh,
                in_specs=(PartitionSpec("core"),) * (n_params + n_outs),
                out_specs=(PartitionSpec("core"),) * n_outs,
                check_rep=False,
            ),
            donate_argnums=tuple(range(n_params, n_params + n_outs)),
            keep_unused=True,
        )
        _RUNNER = (sharded, list(in_names), list(out_names), list(out_avals))
    return _RUNNER


def pack_biases(inputs):
    """Host-side bias staging: stripe gate biases to [P, 6*hsub] (including
    negated z-biases) and broadcast bfc to [P, DOUT]."""
    hsub = H // P

    def stripe(name):
        return np.asarray(inputs[name], np.float32).reshape(hsub, P).T

    pack = np.concatenate(
        [
            stripe("bz0"), stripe("bh0"), stripe("bz1"), stripe("bh1"),
            -stripe("bz0"), -stripe("bz1"),
        ],
        axis=1,
    )
    bfc_rep = np.broadcast_to(
        np.asarray(inputs["bfc"], np.float32), (P, DOUT)
    )
    return {
        "bias_pack": np.ascontiguousarray(pack),
        "bfc_rep": np.ascontiguousarray(bfc_rep),
    }


def run(inputs, trace=False, **spmd_kwargs):
    """Run the SPMD kernel on all 8 cores. Returns (y[B,T,DOUT], results)."""
    x = np.asarray(inputs["x"], dtype=np.float32)
    assert x.shape == (B, T, DIN), x.shape
    # [B, T, DIN] -> per-core [DIN/P, P, T] bf16 (cast + transpose staging)
    x_bf = np.ascontiguousarray(
        x.astype(ml_dtypes.bfloat16).transpose(0, 2, 1).reshape(B, DIN // P, P, T)
    )
    # fp8 copy of features 0..255, [B, P, 2, T], pair-sliceable on chip
    x_f8 = np.ascontiguousarray(
        x[:, :, : 2 * P]
        .astype(ml_dtypes.float8_e4m3)
        .transpose(0, 2, 1)
        .reshape(B, 2, P, T)
        .transpose(0, 2, 1, 3)
    )
    shared = {}
    for name in GEMM_W:
        host_dt = ml_dtypes.float8_e4m3 if name == "Wz1" else ml_dtypes.bfloat16
        w = np.asarray(inputs[name], dtype=np.float32).astype(host_dt)
        if name != "Wfc":
            # [K, N] -> m-major [P, m, k, 128]
            kd, nd = w.shape
            w = w.reshape(kd // P, P, nd // P, P).transpose(1, 2, 0, 3)
        shared[name] = np.ascontiguousarray(w)
    wz0f8 = np.asarray(inputs["Wz0"], dtype=np.float32)[: 2 * P].astype(
        ml_dtypes.float8_e4m3
    )
    shared["Wz0f8"] = np.ascontiguousarray(
        wz0f8.reshape(2, P, H // P, P).transpose(1, 2, 0, 3)
    )
    shared.update(pack_biases(inputs))
    shared["warm_in"] = np.zeros((P, P), dtype=ml_dtypes.bfloat16)
    in_maps = [dict(shared, xT=x_bf[c], xT8=x_f8[c]) for c in range(B)]
    if trace or spmd_kwargs:
        nc = _get_nc()
        res = bass_utils.run_bass_kernel_spmd(
            nc, in_maps, core_ids=list(range(B)), trace=trace, **spmd_kwargs
        )
        y = np.stack([r["y"] for r in res.results], axis=0).astype(np.float32)
        return y, res
    sharded, in_names, out_names, out_avals = _get_runner()
    per_core = [[np.asarray(m[n]) for n in in_names] for m in in_maps]
    concat_in = [
        np.concatenate([per_core[c][i] for c in range(B)], axis=0)
        for i in range(len(in_names))
    ]
    concat_zeros = [
        np.zeros((B * a.shape[0], *a.shape[1:]), a.dtype) for a in out_avals
    ]
    outs = sharded(*concat_in, *concat_zeros)
    yi = out_names.index("y")
    y = np.asarray(outs[yi]).reshape(B, *out_avals[yi].shape).astype(np.float32)
    return y, None


def kernel(**inputs) -> np.ndarray:
    y, _ = run(inputs)
    return y

